# revision 27
# baseline (speedup 1.0000x reference)
"""BertLexer Trainium2 kernel.

Computes, for full inputs
    word_indices [16,256] int, span_start/span_end [16,256] int,
    W_embed [50002,256] f32, hidden_states [12,16,512,768] f32
the reference
    word_emb = W_embed[word_indices]                                # [B,W,E]
    bert_sub = hidden_states.mean(axis=0)                           # [B,S,H]
    bert_emb[b,w] = mean(bert_sub[b, span_start:span_end])          # [B,W,H]
    out = concat([word_emb, bert_emb], axis=2)                      # [B,W,E+H]

Strategy: data-parallel over the batch dim across 8 NeuronCores (2 batches
per core).  Only subwords below max(span_end) are ever referenced, so the
host slices hidden_states to SP = 384+T rows per batch before staging.
Per (layer, batch) one flat DMA [128, 2304] (partition p = subwords
3p..3p+2, 9216B contiguous runs) streams on the sync queue, and a packed
tail DMA [128, 6T] (subwords 384..SP flattened) on the gpsimd queue.  The
12-layer sum is fully decoupled: the main 2304-col chain runs alone on the
DVE; the tiny tail chain, the span-selection masks and the word-embedding
gathers run on GpSimd.  The packed tail sum is un-flattened to [T, 768]
through a DRAM bounce and contracted by a T-partition matmul that opens
the PSUM accumulation before the main stream ends.  Span selection uses an
on-chip mask M[w,s] = (start_w <= s < end_w)/(12*len_w) built against a
layout-matched iota, PE-transposed, and contracted with fp32r matmuls.
PSUM drains via ACT into the output row tile; one 512KB store per w-tile.
HBM-bound on the hidden_states read.
"""

import sys

import numpy as np

if "/opt/trn_rl_repo" not in sys.path:
    sys.path.insert(0, "/opt/trn_rl_repo")

import concourse.bacc as bacc
import concourse.bass as bass
import concourse.mybir as mybir
import concourse.tile as tile
from concourse.masks import make_identity

B, W, S, H, L, E, V = 16, 256, 512, 768, 12, 256, 50002
NCORES = 8
BPC = B // NCORES  # batches per core
P = 128
WT = W // P  # word-index tiles per batch
SFULL = 3 * P  # subwords covered by the full-region tiles (s = 3p + j)
CF = 3 * H  # full-region tile cols (2304)
NCHUNKS = [(0, 512), (512, 256)]  # PSUM-bank-sized pieces of H

F32 = mybir.dt.float32
I32 = mybir.dt.int32

USE_ACCUM_DMA = False  # sum the 12 layers inside gpsimd (software-DGE) DMAs


def build_program(T):
    """T = tail subword count (power of two <= 128, or 0). SP = 384 + T."""
    SP = SFULL + T
    CT = (T * H) // P  # packed tail cols (6T)
    SPM = SFULL + T  # mask columns
    nc = bacc.Bacc(
        "TRN2", target_bir_lowering=False, debug=False, num_devices=NCORES
    )
    wi = nc.dram_tensor("word_indices", [BPC, W], I32, kind="ExternalInput").ap()
    ss = nc.dram_tensor("span_start", [BPC, W], I32, kind="ExternalInput").ap()
    se = nc.dram_tensor("span_end", [BPC, W], I32, kind="ExternalInput").ap()
    emb = nc.dram_tensor("W_embed", [V, E], F32, kind="ExternalInput").ap()
    hs = nc.dram_tensor("hidden_states", [L, BPC, SP * H], F32, kind="ExternalInput").ap()
    out = nc.dram_tensor("out", [BPC, W, E + H], F32, kind="ExternalOutput").ap()
    tsc = (
        nc.dram_tensor("tail_scratch", [BPC, T * H], F32, kind="Internal").ap()
        if T
        else None
    )

    with tile.TileContext(nc) as tc:
        with (
            tc.tile_pool(name="const", bufs=1) as const_pool,
            tc.tile_pool(name="idx", bufs=2) as idx_pool,
            tc.tile_pool(name="mask", bufs=1) as mask_pool,
            tc.tile_pool(name="maskT", bufs=2) as maskT_pool,
            tc.tile_pool(name="hbuf", bufs=15) as h_pool,
            tc.tile_pool(name="htail", bufs=16) as ht_pool,
            tc.tile_pool(name="hsum", bufs=2) as hsum_pool,
            tc.tile_pool(name="tailb", bufs=2) as tail_pool,
            tc.tile_pool(name="obuf", bufs=4) as o_pool,
            tc.tile_pool(name="ptr", bufs=2, space="PSUM") as ptr_pool,
            tc.tile_pool(name="pout", bufs=1, space="PSUM") as pout_pool,
        ):
            identity = const_pool.tile([P, P], F32)
            make_identity(nc, identity)
            # iota column c holds the subword index mapped to mask column c:
            # cols j*128+p (j<3) -> 3p+j; cols 384.. -> 384..SP-1 (tail).
            iota_i = const_pool.tile([P, SPM], I32)
            nc.gpsimd.iota(
                iota_i[:, 0:SFULL], pattern=[[1, 3], [3, P]], base=0,
                channel_multiplier=0,
            )
            if T:
                nc.gpsimd.iota(
                    iota_i[:, SFULL:SPM], pattern=[[1, T]], base=SFULL,
                    channel_multiplier=0,
                )
            iota_f = const_pool.tile([P, SPM], F32)
            nc.gpsimd.tensor_copy(iota_f, iota_i)

            # --- batched index loads up front: [128, BPC*WT] per tensor,
            # column (b*WT + wt) holds word-tile wt of batch b ---
            ss_i = idx_pool.tile([P, BPC * WT], I32, bufs=1)
            se_i = idx_pool.tile([P, BPC * WT], I32, bufs=1)
            wi_i = idx_pool.tile([P, BPC * WT], I32, bufs=1)
            for b in range(BPC):
                csl = slice(b * WT, (b + 1) * WT)
                nc.scalar.dma_start(
                    out=ss_i[:, csl], in_=ss[b, :].rearrange("(w p) -> p w", p=P)
                )
                nc.scalar.dma_start(
                    out=se_i[:, csl], in_=se[b, :].rearrange("(w p) -> p w", p=P)
                )
                nc.scalar.dma_start(
                    out=wi_i[:, csl], in_=wi[b, :].rearrange("(w p) -> p w", p=P)
                )
            ss_f = idx_pool.tile([P, BPC * WT], F32, bufs=1)
            se_f = idx_pool.tile([P, BPC * WT], F32, bufs=1)
            scale = idx_pool.tile([P, BPC * WT], F32, bufs=1)
            nc.vector.tensor_copy(ss_f, ss_i)
            nc.vector.tensor_copy(se_f, se_i)
            len_f = idx_pool.tile([P, BPC * WT], F32, bufs=1)
            nc.vector.tensor_tensor(len_f, se_f, ss_f, op=mybir.AluOpType.subtract)
            rlen = idx_pool.tile([P, BPC * WT], F32, bufs=1)
            nc.vector.reciprocal(rlen, len_f)
            nc.vector.tensor_scalar_mul(scale, rlen, 1.0 / L)

            for b in range(BPC):
                # --- h streaming.  accum mode: every layer DMA adds straight
                # into the running sum inside the DMA engines (software DGE);
                # no SBUF layer ring, no DVE adds.  Copy mode: per-layer tiles
                # on dual HW queues + DVE chain. ---
                h_bigs, h_tails = [], []
                hsum_t = hsum_pool.tile([P, CF + 32], F32, tag="hsum", name=f"hs_{b}")
                hsum = hsum_t[:, 0:CF]
                hsumt = None
                if T:
                    hsumt = hsum_pool.tile([P, CT], F32, tag="hsumt", name=f"hst_{b}")
                if USE_ACCUM_DMA:
                    nc.gpsimd.memset(hsum_t, 0.0)
                    if T:
                        nc.gpsimd.memset(hsumt, 0.0)
                    for l in range(L):
                        nc.gpsimd.dma_start(
                            out=hsum,
                            in_=hs[l, b, 0 : SFULL * H].rearrange(
                                "(p x) -> p x", p=P
                            ),
                            accum_op=mybir.AluOpType.add,
                        )
                        if T:
                            nc.gpsimd.dma_start(
                                out=hsumt,
                                in_=hs[l, b, SFULL * H : SP * H].rearrange(
                                    "(p x) -> p x", p=P
                                ),
                                accum_op=mybir.AluOpType.add,
                            )
                else:
                    for l in range(L):
                        if T:
                            htl = ht_pool.tile(
                                [P, CT], F32, tag="ht", name=f"ht_{b}_{l}"
                            )
                            nc.gpsimd.dma_start(
                                out=htl,
                                in_=hs[l, b, SFULL * H : SP * H].rearrange(
                                    "(p x) -> p x", p=P
                                ),
                            )
                            h_tails.append(htl)
                        hb = h_pool.tile(
                            [P, CF + 32], F32, tag="h", name=f"h_{b}_{l}"
                        )
                        big_eng = nc.sync if l % 2 == 0 else nc.scalar
                        big_eng.dma_start(
                            out=hb[:, 0:CF],
                            in_=hs[l, b, 0 : SFULL * H].rearrange(
                                "(p x) -> p x", p=P
                            ),
                        )
                        h_bigs.append(hb)

                # --- span-selection masks (GpSimd) + PE transposes ---
                maskT_full = maskT_pool.tile([P, 3 * W], F32, tag="mtf")
                maskT_tail = None
                if T:
                    maskT_tail = maskT_pool.tile(
                        [T, W], F32, tag="mtt", name=f"mtt_{b}"
                    )
                for wt in range(WT):
                    c = b * WT + wt
                    m1 = mask_pool.tile([P, SPM], F32, tag="m1")
                    nc.vector.tensor_scalar(
                        m1,
                        iota_f,
                        scalar1=ss_f[:, c : c + 1],
                        scalar2=scale[:, c : c + 1],
                        op0=mybir.AluOpType.is_ge,
                        op1=mybir.AluOpType.mult,
                    )
                    m2 = mask_pool.tile([P, SPM], F32, tag="m2")
                    nc.vector.tensor_scalar(
                        m2,
                        iota_f,
                        scalar1=se_f[:, c : c + 1],
                        scalar2=None,
                        op0=mybir.AluOpType.is_lt,
                    )
                    mM = mask_pool.tile([P, SPM], F32, tag="mM")
                    nc.vector.tensor_tensor(mM, m1, m2, op=mybir.AluOpType.mult)
                    for j in range(3):
                        ptr = ptr_pool.tile([P, P], F32, space="PSUM", tag="ptr")
                        nc.tensor.transpose(
                            ptr, mM[:, j * P : (j + 1) * P], identity
                        )
                        col = (j * WT + wt) * P
                        nc.scalar.copy(maskT_full[:, col : col + P], ptr)
                    if T:
                        ptrT = ptr_pool.tile([T, P], F32, space="PSUM", tag="ptrT")
                        nc.tensor.transpose(ptrT, mM[:, SFULL:SPM], identity)
                        nc.scalar.copy(
                            maskT_tail[:, wt * P : (wt + 1) * P], ptrT
                        )

                # --- word-embedding gather lands directly in the row tile ---
                obufs = []
                for wt in range(WT):
                    obuf = o_pool.tile(
                        [P, E + H], F32, tag="obuf", name=f"obuf_{b}_{wt}"
                    )
                    nc.gpsimd.indirect_dma_start(
                        out=obuf[:, 0:E],
                        out_offset=None,
                        in_=emb[:, :],
                        in_offset=bass.IndirectOffsetOnAxis(
                            ap=wi_i[:, b * WT + wt : b * WT + wt + 1], axis=0
                        ),
                    )
                    obufs.append(obuf)

                # --- exact f32 12-layer sum: main chain alone on the DVE,
                # packed tail chain on GpSimd ---
                PE_LAYERS = ()
                tail16 = None
                if not USE_ACCUM_DMA:
                    PE_LAYERS = ()  # PE keeps only span matmuls
                    # tail chain first: its DMAs land early, so the sum, the
                    # DRAM-bounce unpack and the tail matmul all complete
                    # while the big stream is still arriving
                    if T:
                        nc.vector.tensor_tensor(
                            hsumt, h_tails[0], h_tails[1], op=mybir.AluOpType.add
                        )
                        for l in range(2, L):
                            nc.vector.tensor_tensor(
                                hsumt, hsumt, h_tails[l], op=mybir.AluOpType.add
                            )
                        # un-flatten the packed tail sum to [T, 768] via a
                        # DRAM bounce (SBUF APs cannot regroup partitions)
                        nc.scalar.dma_start(
                            out=tsc[b, :].rearrange("(p x) -> p x", p=P), in_=hsumt
                        )
                        tail16 = tail_pool.tile(
                            [T, H], F32, tag="t16", name=f"t16_{b}"
                        )
                        nc.scalar.dma_start(
                            out=tail16, in_=tsc[b, :].rearrange("(t x) -> t x", t=T)
                        )
                    if b < BPC - 1:
                        nc.vector.tensor_tensor(
                            hsum, h_bigs[0][:, 0:CF], h_bigs[1][:, 0:CF],
                            op=mybir.AluOpType.add,
                        )
                        for l in range(2, L):
                            nc.vector.tensor_tensor(
                                hsum, hsum, h_bigs[l][:, 0:CF],
                                op=mybir.AluOpType.add,
                            )
                    else:
                        # last batch: three per-j chains so the final adds
                        # pipeline with the span matmuls after the last layer
                        for l in range(1, L):
                            for j in range(3):
                                jsl = slice(j * H, (j + 1) * H)
                                nc.vector.tensor_tensor(
                                    hsum[:, jsl],
                                    h_bigs[0][:, jsl] if l == 1 else hsum[:, jsl],
                                    h_bigs[l][:, jsl],
                                    op=mybir.AluOpType.add,
                                )
                elif T:
                    # un-flatten the packed tail sum to [T, 768] via a DRAM
                    # bounce (SBUF APs cannot regroup the partition dim)
                    nc.scalar.dma_start(
                        out=tsc[b, :].rearrange("(p x) -> p x", p=P), in_=hsumt
                    )
                    tail16 = tail_pool.tile([T, H], F32, tag="t16", name=f"t16_{b}")
                    nc.scalar.dma_start(
                        out=tail16, in_=tsc[b, :].rearrange("(t x) -> t x", t=T)
                    )

                # --- span matmuls, phase-ordered so mid-stream work never
                # sits behind the late tail16 dependency on the PE:
                # phase 1: per-layer MMs for BOTH w-tiles (h arrives early),
                # phase 2: tail MMs, phase 3: hsum MMs (chain-end). ---
                pouts = []
                for wt in range(WT):
                    pout = pout_pool.tile(
                        [P, H], F32, space="PSUM", tag=f"pout{wt}",
                        name=f"pout{wt}_{b}",
                    )
                    pouts.append(pout)
                # start=True exactly once per (pout, chunk) accumulation zone:
                # on the first PE layer's j==0 matmul, else on the tail MM,
                # else on the hsum j==0 matmul.
                for l in PE_LAYERS:
                    for wt in range(WT):
                        for j in range(3):
                            col = (j * WT + wt) * P
                            for n0, nl in NCHUNKS:
                                nc.tensor.matmul(
                                    pouts[wt][:, n0 : n0 + nl],
                                    lhsT=maskT_full[:, col : col + P],
                                    rhs=h_bigs[l][:, j * H + n0 : j * H + n0 + nl],
                                    start=(l == PE_LAYERS[0] and j == 0),
                                    stop=False,
                                )
                if T:
                    for wt in range(WT):
                        for n0, nl in NCHUNKS:
                            nc.tensor.matmul(
                                pouts[wt][:, n0 : n0 + nl],
                                lhsT=maskT_tail[:, wt * P : (wt + 1) * P],
                                rhs=tail16[:, n0 : n0 + nl],
                                start=not PE_LAYERS,
                                stop=False,
                            )
                # j-major on the last batch so MM j0 fires right after its
                # chain while j1/j2 adds still run
                for j in range(3):
                    for wt in range(WT):
                        col = (j * WT + wt) * P
                        for n0, nl in NCHUNKS:
                            nc.tensor.matmul(
                                pouts[wt][:, n0 : n0 + nl],
                                lhsT=maskT_full[:, col : col + P],
                                rhs=hsum[:, j * H + n0 : j * H + n0 + nl],
                                start=(j == 0 and not PE_LAYERS and not T),
                                stop=(j == 2),
                            )
                for wt in range(WT):
                    # PSUM -> row tile on ACT; store once per w-tile
                    for n0, nl in NCHUNKS:
                        nc.scalar.copy(
                            obufs[wt][:, E + n0 : E + n0 + nl],
                            pouts[wt][:, n0 : n0 + nl],
                        )
                    store_eng = nc.sync if b == BPC - 1 else nc.scalar
                    wsl = slice(wt * P, (wt + 1) * P)
                    store_eng.dma_start(out=out[b, wsl, :], in_=obufs[wt])

    nc.compile()
    return nc


_NC = {}


def _tail_for(s_used):
    """Round the needed tail (beyond 384) up to a power of two <= 128."""
    if s_used <= SFULL:
        return 0
    t = s_used - SFULL
    p = 1
    while p < t:
        p *= 2
    return min(p, P)


def _get_program(T=16):
    if T not in _NC:
        _NC[T] = build_program(T)
    return _NC[T]


def make_in_maps(word_indices, span_start, span_end, W_embed, hidden_states, T):
    SP = SFULL + T
    emb = np.ascontiguousarray(W_embed, dtype=np.float32)
    in_maps = []
    for c in range(NCORES):
        bsl = slice(BPC * c, BPC * (c + 1))
        hsc = np.ascontiguousarray(
            hidden_states[:, bsl, :SP, :], dtype=np.float32
        ).reshape(L, BPC, SP * H)
        in_maps.append(
            {
                "word_indices": np.ascontiguousarray(
                    word_indices[bsl], dtype=np.int32
                ),
                "span_start": np.ascontiguousarray(span_start[bsl], dtype=np.int32),
                "span_end": np.ascontiguousarray(span_end[bsl], dtype=np.int32),
                "W_embed": emb,
                "hidden_states": hsc,
            }
        )
    return in_maps


def run(word_indices, span_start, span_end, W_embed, hidden_states, **run_kwargs):
    from concourse.bass_utils import run_bass_kernel_spmd

    s_used = int(np.max(np.asarray(span_end)[:, -1]))
    T = _tail_for(s_used)
    nc = _get_program(T)
    in_maps = make_in_maps(
        word_indices, span_start, span_end, W_embed, hidden_states, T
    )
    res = run_bass_kernel_spmd(nc, in_maps, core_ids=list(range(NCORES)), **run_kwargs)
    out = np.concatenate([res.results[c]["out"] for c in range(NCORES)], axis=0)
    return out, res


def kernel(word_indices, span_start, span_end, W_embed, hidden_states):
    out, _ = run(word_indices, span_start, span_end, W_embed, hidden_states)
    return out


# revision 28
# speedup vs baseline: 1.1339x; 1.1339x over previous
"""BertLexer Trainium2 kernel.

Computes, for full inputs
    word_indices [16,256] int, span_start/span_end [16,256] int,
    W_embed [50002,256] f32, hidden_states [12,16,512,768] f32
the reference
    word_emb = W_embed[word_indices]                                # [B,W,E]
    bert_sub = hidden_states.mean(axis=0)                           # [B,S,H]
    bert_emb[b,w] = mean(bert_sub[b, span_start:span_end])          # [B,W,H]
    out = concat([word_emb, bert_emb], axis=2)                      # [B,W,E+H]

Strategy: data-parallel over the batch dim across 8 NeuronCores (2 batches
per core).  Only subwords below max(span_end) are ever referenced, so the
host slices hidden_states to SP = 384+T rows per batch before staging.
Per (layer, batch) one flat DMA [128, 2304] (partition p = subwords
3p..3p+2, 9216B contiguous runs) streams on the sync queue, and a packed
tail DMA [128, 6T] (subwords 384..SP flattened) on the gpsimd queue.  The
12-layer sum is fully decoupled: the main 2304-col chain runs alone on the
DVE; the tiny tail chain, the span-selection masks and the word-embedding
gathers run on GpSimd.  The packed tail sum is un-flattened to [T, 768]
through a DRAM bounce and contracted by a T-partition matmul that opens
the PSUM accumulation before the main stream ends.  Span selection uses an
on-chip mask M[w,s] = (start_w <= s < end_w)/(12*len_w) built against a
layout-matched iota, PE-transposed, and contracted with fp32r matmuls.
PSUM drains via ACT into the output row tile; one 512KB store per w-tile.
HBM-bound on the hidden_states read.
"""

import sys

import numpy as np

if "/opt/trn_rl_repo" not in sys.path:
    sys.path.insert(0, "/opt/trn_rl_repo")

import concourse.bacc as bacc
import concourse.bass as bass
import concourse.mybir as mybir
import concourse.tile as tile
from concourse.masks import make_identity

B, W, S, H, L, E, V = 16, 256, 512, 768, 12, 256, 50002
NCORES = 8
BPC = B // NCORES  # batches per core
P = 128
WT = W // P  # word-index tiles per batch
SFULL = 3 * P  # subwords covered by the full-region tiles (s = 3p + j)
CF = 3 * H  # full-region tile cols (2304)
NCHUNKS = [(0, 512), (512, 256)]  # PSUM-bank-sized pieces of H

F32 = mybir.dt.float32
I32 = mybir.dt.int32

USE_ACCUM_DMA = False  # sum the 12 layers inside gpsimd (software-DGE) DMAs


def build_program(T):
    """T = tail subword count (power of two <= 128, or 0). SP = 384 + T."""
    SP = SFULL + T
    CT = (T * H) // P  # packed tail cols (6T)
    SPM = SFULL + T  # mask columns
    nc = bacc.Bacc(
        "TRN2", target_bir_lowering=False, debug=False, num_devices=NCORES
    )
    wi = nc.dram_tensor("word_indices", [BPC, W], I32, kind="ExternalInput").ap()
    ss = nc.dram_tensor("span_start", [BPC, W], I32, kind="ExternalInput").ap()
    se = nc.dram_tensor("span_end", [BPC, W], I32, kind="ExternalInput").ap()
    emb = nc.dram_tensor("W_embed", [V, E], F32, kind="ExternalInput").ap()
    hs = nc.dram_tensor("hidden_states", [L, BPC, SP * H], F32, kind="ExternalInput").ap()
    out = nc.dram_tensor("out", [BPC, W, E + H], F32, kind="ExternalOutput").ap()
    tsc = (
        nc.dram_tensor("tail_scratch", [BPC, T * H], F32, kind="Internal").ap()
        if T
        else None
    )

    with tile.TileContext(nc) as tc:
        with (
            tc.tile_pool(name="const", bufs=1) as const_pool,
            tc.tile_pool(name="idx", bufs=2) as idx_pool,
            tc.tile_pool(name="mask", bufs=2) as mask_pool,
            tc.tile_pool(name="maskT", bufs=2) as maskT_pool,
            tc.tile_pool(name="hbuf", bufs=15) as h_pool,
            tc.tile_pool(name="htail", bufs=16) as ht_pool,
            tc.tile_pool(name="hsum", bufs=2) as hsum_pool,
            tc.tile_pool(name="tailb", bufs=2) as tail_pool,
            tc.tile_pool(name="obuf", bufs=4) as o_pool,
            tc.tile_pool(name="ptr", bufs=2, space="PSUM") as ptr_pool,
            tc.tile_pool(name="pout", bufs=1, space="PSUM") as pout_pool,
        ):
            identity = const_pool.tile([P, P], F32)
            make_identity(nc, identity)
            # iota column c holds the subword index mapped to mask column c:
            # cols j*128+p (j<3) -> 3p+j; cols 384.. -> 384..SP-1 (tail).
            iota_i = const_pool.tile([P, SPM], I32)
            nc.gpsimd.iota(
                iota_i[:, 0:SFULL], pattern=[[1, 3], [3, P]], base=0,
                channel_multiplier=0,
            )
            if T:
                nc.gpsimd.iota(
                    iota_i[:, SFULL:SPM], pattern=[[1, T]], base=SFULL,
                    channel_multiplier=0,
                )
            iota_f = const_pool.tile([P, SPM], F32)
            nc.gpsimd.tensor_copy(iota_f, iota_i)

            # --- batched index loads up front: [128, BPC*WT] per tensor,
            # column (b*WT + wt) holds word-tile wt of batch b ---
            ss_i = idx_pool.tile([P, BPC * WT], I32, bufs=1)
            se_i = idx_pool.tile([P, BPC * WT], I32, bufs=1)
            wi_i = idx_pool.tile([P, BPC * WT], I32, bufs=1)
            for b in range(BPC):
                csl = slice(b * WT, (b + 1) * WT)
                nc.scalar.dma_start(
                    out=ss_i[:, csl], in_=ss[b, :].rearrange("(w p) -> p w", p=P)
                )
                nc.scalar.dma_start(
                    out=se_i[:, csl], in_=se[b, :].rearrange("(w p) -> p w", p=P)
                )
                nc.scalar.dma_start(
                    out=wi_i[:, csl], in_=wi[b, :].rearrange("(w p) -> p w", p=P)
                )
            ss_f = idx_pool.tile([P, BPC * WT], F32, bufs=1)
            se_f = idx_pool.tile([P, BPC * WT], F32, bufs=1)
            scale = idx_pool.tile([P, BPC * WT], F32, bufs=1)
            nc.vector.tensor_copy(ss_f, ss_i)
            nc.vector.tensor_copy(se_f, se_i)
            len_f = idx_pool.tile([P, BPC * WT], F32, bufs=1)
            nc.vector.tensor_tensor(len_f, se_f, ss_f, op=mybir.AluOpType.subtract)
            rlen = idx_pool.tile([P, BPC * WT], F32, bufs=1)
            nc.vector.reciprocal(rlen, len_f)
            nc.vector.tensor_scalar_mul(scale, rlen, 1.0 / L)

            for b in range(BPC):
                # --- h streaming.  accum mode: every layer DMA adds straight
                # into the running sum inside the DMA engines (software DGE);
                # no SBUF layer ring, no DVE adds.  Copy mode: per-layer tiles
                # on dual HW queues + DVE chain. ---
                h_bigs, h_tails = [], []
                hsum_t = hsum_pool.tile([P, CF + 32], F32, tag="hsum", name=f"hs_{b}")
                hsum = hsum_t[:, 0:CF]
                hsumt = None
                if T:
                    hsumt = hsum_pool.tile([P, CT], F32, tag="hsumt", name=f"hst_{b}")
                if USE_ACCUM_DMA:
                    nc.gpsimd.memset(hsum_t, 0.0)
                    if T:
                        nc.gpsimd.memset(hsumt, 0.0)
                    for l in range(L):
                        nc.gpsimd.dma_start(
                            out=hsum,
                            in_=hs[l, b, 0 : SFULL * H].rearrange(
                                "(p x) -> p x", p=P
                            ),
                            accum_op=mybir.AluOpType.add,
                        )
                        if T:
                            nc.gpsimd.dma_start(
                                out=hsumt,
                                in_=hs[l, b, SFULL * H : SP * H].rearrange(
                                    "(p x) -> p x", p=P
                                ),
                                accum_op=mybir.AluOpType.add,
                            )
                else:
                    for l in range(L):
                        if T:
                            htl = ht_pool.tile(
                                [P, CT], F32, tag="ht", name=f"ht_{b}_{l}"
                            )
                            nc.gpsimd.dma_start(
                                out=htl,
                                in_=hs[l, b, SFULL * H : SP * H].rearrange(
                                    "(p x) -> p x", p=P
                                ),
                            )
                            h_tails.append(htl)
                        hb = h_pool.tile(
                            [P, CF + 32], F32, tag="h", name=f"h_{b}_{l}"
                        )
                        big_eng = nc.sync if l % 2 == 0 else nc.scalar
                        big_eng.dma_start(
                            out=hb[:, 0:CF],
                            in_=hs[l, b, 0 : SFULL * H].rearrange(
                                "(p x) -> p x", p=P
                            ),
                        )
                        h_bigs.append(hb)

                # --- span-selection masks (GpSimd) + PE transposes ---
                maskT_full = maskT_pool.tile([P, 3 * W], F32, tag="mtf")
                maskT_tail = None
                if T:
                    maskT_tail = maskT_pool.tile(
                        [T, W], F32, tag="mtt", name=f"mtt_{b}"
                    )
                for wt in range(WT):
                    c = b * WT + wt
                    m1 = mask_pool.tile([P, SPM], F32, tag="m1")
                    nc.vector.tensor_scalar(
                        m1,
                        iota_f,
                        scalar1=ss_f[:, c : c + 1],
                        scalar2=scale[:, c : c + 1],
                        op0=mybir.AluOpType.is_ge,
                        op1=mybir.AluOpType.mult,
                    )
                    m2 = mask_pool.tile([P, SPM], F32, tag="m2")
                    nc.vector.tensor_scalar(
                        m2,
                        iota_f,
                        scalar1=se_f[:, c : c + 1],
                        scalar2=None,
                        op0=mybir.AluOpType.is_lt,
                    )
                    mM = mask_pool.tile([P, SPM], F32, tag="mM")
                    nc.vector.tensor_tensor(mM, m1, m2, op=mybir.AluOpType.mult)
                    for j in range(3):
                        ptr = ptr_pool.tile([P, P], F32, space="PSUM", tag="ptr")
                        nc.tensor.transpose(
                            ptr, mM[:, j * P : (j + 1) * P], identity
                        )
                        col = (j * WT + wt) * P
                        nc.scalar.copy(maskT_full[:, col : col + P], ptr)
                    if T:
                        ptrT = ptr_pool.tile([T, P], F32, space="PSUM", tag="ptrT")
                        nc.tensor.transpose(ptrT, mM[:, SFULL:SPM], identity)
                        nc.scalar.copy(
                            maskT_tail[:, wt * P : (wt + 1) * P], ptrT
                        )

                # --- word-embedding gather lands directly in the row tile ---
                obufs = []
                for wt in range(WT):
                    obuf = o_pool.tile(
                        [P, E + H], F32, tag="obuf", name=f"obuf_{b}_{wt}"
                    )
                    nc.gpsimd.indirect_dma_start(
                        out=obuf[:, 0:E],
                        out_offset=None,
                        in_=emb[:, :],
                        in_offset=bass.IndirectOffsetOnAxis(
                            ap=wi_i[:, b * WT + wt : b * WT + wt + 1], axis=0
                        ),
                    )
                    obufs.append(obuf)

                # --- exact f32 12-layer sum: main chain alone on the DVE,
                # packed tail chain on GpSimd ---
                PE_LAYERS = ()
                tail16 = None
                if not USE_ACCUM_DMA:
                    PE_LAYERS = ()  # PE keeps only span matmuls
                    # tail chain first: its DMAs land early, so the sum, the
                    # DRAM-bounce unpack and the tail matmul all complete
                    # while the big stream is still arriving
                    if T:
                        nc.vector.tensor_tensor(
                            hsumt, h_tails[0], h_tails[1], op=mybir.AluOpType.add
                        )
                        for l in range(2, L):
                            nc.vector.tensor_tensor(
                                hsumt, hsumt, h_tails[l], op=mybir.AluOpType.add
                            )
                        # un-flatten the packed tail sum to [T, 768] via a
                        # DRAM bounce (SBUF APs cannot regroup partitions)
                        nc.scalar.dma_start(
                            out=tsc[b, :].rearrange("(p x) -> p x", p=P), in_=hsumt
                        )
                        tail16 = tail_pool.tile(
                            [T, H], F32, tag="t16", name=f"t16_{b}"
                        )
                        nc.scalar.dma_start(
                            out=tail16, in_=tsc[b, :].rearrange("(t x) -> t x", t=T)
                        )
                    nc.vector.tensor_tensor(
                        hsum, h_bigs[0][:, 0:CF], h_bigs[1][:, 0:CF],
                        op=mybir.AluOpType.add,
                    )
                    for l in range(2, L):
                        nc.vector.tensor_tensor(
                            hsum, hsum, h_bigs[l][:, 0:CF],
                            op=mybir.AluOpType.add,
                        )
                elif T:
                    # un-flatten the packed tail sum to [T, 768] via a DRAM
                    # bounce (SBUF APs cannot regroup the partition dim)
                    nc.scalar.dma_start(
                        out=tsc[b, :].rearrange("(p x) -> p x", p=P), in_=hsumt
                    )
                    tail16 = tail_pool.tile([T, H], F32, tag="t16", name=f"t16_{b}")
                    nc.scalar.dma_start(
                        out=tail16, in_=tsc[b, :].rearrange("(t x) -> t x", t=T)
                    )

                # --- span matmuls, phase-ordered so mid-stream work never
                # sits behind the late tail16 dependency on the PE:
                # phase 1: per-layer MMs for BOTH w-tiles (h arrives early),
                # phase 2: tail MMs, phase 3: hsum MMs (chain-end). ---
                pouts = []
                for wt in range(WT):
                    pout = pout_pool.tile(
                        [P, H], F32, space="PSUM", tag=f"pout{wt}",
                        name=f"pout{wt}_{b}",
                    )
                    pouts.append(pout)
                # start=True exactly once per (pout, chunk) accumulation zone:
                # on the first PE layer's j==0 matmul, else on the tail MM,
                # else on the hsum j==0 matmul.
                for l in PE_LAYERS:
                    for wt in range(WT):
                        for j in range(3):
                            col = (j * WT + wt) * P
                            for n0, nl in NCHUNKS:
                                nc.tensor.matmul(
                                    pouts[wt][:, n0 : n0 + nl],
                                    lhsT=maskT_full[:, col : col + P],
                                    rhs=h_bigs[l][:, j * H + n0 : j * H + n0 + nl],
                                    start=(l == PE_LAYERS[0] and j == 0),
                                    stop=False,
                                )
                if T:
                    for wt in range(WT):
                        for n0, nl in NCHUNKS:
                            nc.tensor.matmul(
                                pouts[wt][:, n0 : n0 + nl],
                                lhsT=maskT_tail[:, wt * P : (wt + 1) * P],
                                rhs=tail16[:, n0 : n0 + nl],
                                start=not PE_LAYERS,
                                stop=False,
                            )
                for wt in range(WT):
                    for j in range(3):
                        col = (j * WT + wt) * P
                        for n0, nl in NCHUNKS:
                            nc.tensor.matmul(
                                pouts[wt][:, n0 : n0 + nl],
                                lhsT=maskT_full[:, col : col + P],
                                rhs=hsum[:, j * H + n0 : j * H + n0 + nl],
                                start=(j == 0 and not PE_LAYERS and not T),
                                stop=(j == 2),
                            )
                    # PSUM -> row tile on ACT; store once per w-tile
                    for n0, nl in NCHUNKS:
                        nc.scalar.copy(
                            obufs[wt][:, E + n0 : E + n0 + nl],
                            pouts[wt][:, n0 : n0 + nl],
                        )
                    store_eng = nc.sync if b == BPC - 1 else nc.scalar
                    wsl = slice(wt * P, (wt + 1) * P)
                    store_eng.dma_start(out=out[b, wsl, :], in_=obufs[wt])

    nc.compile()
    return nc


_NC = {}


def _tail_for(s_used):
    """Round the needed tail (beyond 384) up to a power of two <= 128."""
    if s_used <= SFULL:
        return 0
    t = s_used - SFULL
    p = 1
    while p < t:
        p *= 2
    return min(p, P)


def _get_program(T=16):
    if T not in _NC:
        _NC[T] = build_program(T)
    return _NC[T]


def make_in_maps(word_indices, span_start, span_end, W_embed, hidden_states, T):
    SP = SFULL + T
    emb = np.ascontiguousarray(W_embed, dtype=np.float32)
    in_maps = []
    for c in range(NCORES):
        bsl = slice(BPC * c, BPC * (c + 1))
        hsc = np.ascontiguousarray(
            hidden_states[:, bsl, :SP, :], dtype=np.float32
        ).reshape(L, BPC, SP * H)
        in_maps.append(
            {
                "word_indices": np.ascontiguousarray(
                    word_indices[bsl], dtype=np.int32
                ),
                "span_start": np.ascontiguousarray(span_start[bsl], dtype=np.int32),
                "span_end": np.ascontiguousarray(span_end[bsl], dtype=np.int32),
                "W_embed": emb,
                "hidden_states": hsc,
            }
        )
    return in_maps


def run(word_indices, span_start, span_end, W_embed, hidden_states, **run_kwargs):
    from concourse.bass_utils import run_bass_kernel_spmd

    s_used = int(np.max(np.asarray(span_end)[:, -1]))
    T = _tail_for(s_used)
    nc = _get_program(T)
    in_maps = make_in_maps(
        word_indices, span_start, span_end, W_embed, hidden_states, T
    )
    res = run_bass_kernel_spmd(nc, in_maps, core_ids=list(range(NCORES)), **run_kwargs)
    out = np.concatenate([res.results[c]["out"] for c in range(NCORES)], axis=0)
    return out, res


def kernel(word_indices, span_start, span_end, W_embed, hidden_states):
    out, _ = run(word_indices, span_start, span_end, W_embed, hidden_states)
    return out
